# revision 2
# baseline (speedup 1.0000x reference)
"""MinGRU Trainium2 kernel — fused custom-DVE scan, bf16 psT + fp8 psK.

Reference computation (per batch element b, sequence length T, hidden H):
    k  = x @ W_z + b_z                       # [T, H]
    th = x @ W_h + b_h                       # [T, H]
    z  = sigmoid(k);  a = 1 - z
    g  = max(th + 0.5, sigmoid(th))          # == where(th>=0, th+0.5, sig(th))
    h[t] = a[t] * h[t-1] + z[t] * g[t]       # linear scan along T
Output h  # [B, T, H]

Data-parallel over batch (B=8 -> 8 NeuronCores). Per core, per chunk
[128 hidden x 1024 time]:
  PE  : psT = x@W_h            (bf16: exact path — th errors reach h 1:1)
        psK = 16*(x@W_z)       (fp8e4 DoubleRow: z goes through a sigmoid,
                                which compresses the quantization error 4x)
  Act : sg = sigmoid(psT + b_h)        -> zs[:,1,:]  (bf16 SBUF)
        z  = sigmoid(psK/16 + b_z)     -> zs[:,0,:]
  DVE : ONE custom op (MINGRU_SCAN_ANT) streams interleaved (z_t, sg_t)
        pairs plus psT and computes w5=psT+(b_h+0.5) [read from a persistent
        swap flop], a=1-z, g=max(w5,sg), b=z*g AND the affine scan
        h[t]=a*h[t-1]+b at 1 element/cycle via a COUNT-alternating uop pair
        (zero-latency transitions; the first uop's write port is disabled so
        the packed [128,1024] bf16 output advances only on the second).

Cross-chunk chaining uses DVE state that persists across instructions:
  - MINGRU_LDBH_ANT (one per m-tile) latches b_h+0.5 into block-0's swap
    flop and zeroes block-3's CURR flop (the h state).
  - The scan op has NO init uop: block-3's flop carries h across the four
    chunk instructions of an m-tile (DVE executes its queue in order).
This removes the K=1 bias matmuls, the f32 h copy, and the init bubble.
"""

import numpy as np

B, T, D, H = 8, 4096, 512, 512
N_CORES = 8
MMN = 512                 # matmul free dim (PSUM bank limit for fp32)
TCH = 1024                # chunk along T
NT = T // TCH             # 4
NM = H // 128             # 4 partition tiles of H
NK = D // 128             # 4 contraction tiles
WSC = 16.0                # fp8 weight scale for the z path

_cache = {}

SCAN_OPS = ["MINGRU_SCAN%d_ANT" % m for m in range(NM)]
LDBH_OP = "MINGRU_LDBH_ANT"


def _register_dve_ops():
    """Register (idempotently) the scan + bh-preload ops; return (scan, ld)."""
    from dataclasses import dataclass

    from concourse import dve_ops as DOPS
    from concourse.dve_spec import C0, Spec, Src0, Src1
    from concourse.dve_uop import (
        DISABLE,
        ENABLE,
        AluInp,
        AluOp,
        DelayInp,
        DveOpSpec,
        InpSel,
        OutPath,
        OutSel,
        Trigger,
        UopConfig,
    )

    have = {op.name: op for op in DOPS.OPS}
    if LDBH_OP in have:
        return [have[n] for n in SCAN_OPS], have[LDBH_OP]
    NM_CONST = NM

    def _bypass_tail(u, start):
        for k in range(start, 8):
            u.datapath_config[k].enable_alu(AluOp.BYPASS, AluInp.PREV_ALU_OUT)

    def _mk_ua(off, next_idx):
        # pair element j=0: Src0=z_t, Src1=psT_t
        #   b[off]:   w5 = psT + swap_off(bh5)   [CURR read by ub's MAX]
        #   b[off+1]: a = 1 - z
        #   b[off+3]: tmp = CURR * a = h_{t-1} * a_t
        ua = UopConfig()
        ua.enable_input(InpSel.SRC_1, 0)
        ua.enable_input(InpSel.SRC_0, 1)    # -> delay chain 0
        ua.enable_input(InpSel.ONE_F32, 2)  # -> delay chain 1
        for k in range(off):
            ua.datapath_config[k].enable_alu(
                AluOp.BYPASS, AluInp.PREV_ALU_OUT).pass_through_delay(0, 1)
        ua.datapath_config[off].enable_alu(
            AluOp.ADD, AluInp.PREV_ALU_OUT, AluInp.CURR_SWAP_OUT
        ).pass_through_delay(0, 1)
        ua.datapath_config[off + 1].enable_alu(
            AluOp.SUBTRACT, AluInp.PREV_DELAY_1, AluInp.PREV_DELAY_0
        )
        ua.datapath_config[off + 2].enable_alu(AluOp.BYPASS,
                                               AluInp.PREV_ALU_OUT)
        ua.datapath_config[off + 3].enable_alu(
            AluOp.MULTIPLY, AluInp.CURR_ALU_OUT, AluInp.PREV_ALU_OUT
        )
        _bypass_tail(ua, off + 4)
        ua.require_inp0 = ENABLE
        ua.require_inp1 = ENABLE
        ua.repeat_count = 1
        ua.trigger = (Trigger.SRC_TENSOR_DONE, Trigger.COUNT, Trigger.NONE)
        ua.next_uop = (0, next_idx, 0)
        return ua

    def _build_scan_uops(off):
        # pair element j=1: Src0=sg_t, Src1 ignored.
        #   b[off]:   g = max(CURR, sg)
        #   b[off+1]: z = 1 - CURR (=1-a)   [capture g into delay chain 0]
        #   b[off+2]: zg = z * g
        #   b[off+3]: h = CURR + zg         [written out]
        ub = UopConfig()
        ub.enable_input(InpSel.SRC_0, 0)
        ub.enable_input(InpSel.ONE_F32, 2)  # -> delay chain 1
        for k in range(off):
            ub.datapath_config[k].enable_alu(
                AluOp.BYPASS, AluInp.PREV_ALU_OUT).pass_through_delay(1)
        ub.datapath_config[off].enable_alu(
            AluOp.MAX, AluInp.CURR_ALU_OUT, AluInp.PREV_ALU_OUT
        ).pass_through_delay(1)
        ub.datapath_config[off + 1].enable_alu(
            AluOp.SUBTRACT, AluInp.PREV_DELAY_1, AluInp.CURR_ALU_OUT
        ).enable_delay_from_src(DelayInp.PREV_ALU_OUT, 0)
        ub.datapath_config[off + 2].enable_alu(
            AluOp.MULTIPLY, AluInp.PREV_ALU_OUT, AluInp.PREV_DELAY_0
        )
        ub.datapath_config[off + 3].enable_alu(
            AluOp.ADD, AluInp.CURR_ALU_OUT, AluInp.PREV_ALU_OUT
        )
        _bypass_tail(ub, off + 4)
        ub.require_inp0 = ENABLE
        ub.require_inp1 = ENABLE
        ub.repeat_count = 1
        ub.trigger = (Trigger.SRC_TENSOR_DONE, Trigger.COUNT, Trigger.NONE)
        ub.next_uop = (0, 1, 0)
        ub.enable_output(OutSel.ALU_OUT, OutPath.WR0_LO)
        # slot0 = INIT (seeds b[off+3]'s CURR with C0 = h_init; the s0 read
        # also hands the scheduler the chunk-to-chunk dependency);
        # slot1 = uopA; slot2 = uopB.
        init = UopConfig()
        init.enable_input(InpSel.CONST_0, 0)
        for k in range(off + 4):
            init.datapath_config[k].enable_alu(AluOp.BYPASS,
                                               AluInp.PREV_ALU_OUT)
        _bypass_tail(init, off + 4)
        init.require_inp0 = DISABLE
        init.require_inp1 = DISABLE
        init.repeat_count = 1
        init.trigger = (Trigger.COUNT, Trigger.NONE, Trigger.NONE)
        init.next_uop = (1, 0, 0)
        return [init, _mk_ua(off, 2), ub]

    def _build_ldbh_uops():
        # Four elements (bh5 for m=0..3, via in0 [P,4]): element m is
        # latched into block-m's swap flop (uop_m runs BYPASS everywhere
        # with swap_enable at block m; the swap flop captures the
        # complementary = B operand as the element passes). Block 7
        # multiplies by zero so the [P,4] output is zeros — it seeds the
        # first scans' s0 and orders them after this op.
        uops = []
        for m in range(NM_CONST):
            ld = UopConfig()
            ld.enable_input(InpSel.SRC_0, 0)
            ld.enable_input(InpSel.ZERO, 2)  # -> delay chain 1
            for k in range(7):
                ld.datapath_config[k].enable_alu(
                    AluOp.BYPASS, AluInp.PREV_ALU_OUT, AluInp.PREV_ALU_OUT
                ).pass_through_delay(1)
            ld.datapath_config[m].swap_enable = ENABLE
            ld.datapath_config[7].enable_alu(
                AluOp.MULTIPLY, AluInp.PREV_ALU_OUT, AluInp.PREV_DELAY_1
            )
            ld.require_inp0 = ENABLE
            ld.require_inp1 = DISABLE
            ld.repeat_count = 1
            if m < NM_CONST - 1:
                ld.trigger = (Trigger.SRC_TENSOR_DONE, Trigger.COUNT,
                              Trigger.NONE)
                ld.next_uop = (0, m + 1, 0)
            else:
                ld.trigger = (Trigger.SRC_TENSOR_DONE, Trigger.NONE,
                              Trigger.NONE)
                ld.next_uop = (0, 0, 0)
            ld.enable_output(OutSel.ALU_OUT, OutPath.WR0_LO)
            uops.append(ld)
        return uops

    def _scan_reference(in0, in1, c0, c1, c2):
        # NOTE: the bh5 swap value and h chaining are cross-instruction
        # state CoreSim can't see; this reference covers a single op with
        # bh5=0, h0=0 (used only by sim paths, not by hardware).
        z = np.asarray(in0[:, :, 0], np.float32)
        sg = np.asarray(in0[:, :, 1], np.float32)
        w5 = np.asarray(in1[:, :, 0], np.float32)
        g = np.maximum(w5, sg)
        bb = z * g
        a = 1.0 - z
        P, N = z.shape
        h = np.empty((P, N), np.float32)
        st = np.zeros((P,), np.float32)
        for t in range(N):
            st = a[:, t] * st + bb[:, t]
            h[:, t] = st
        return h

    def _ld_reference(in0, in1, c0, c1, c2):
        return np.zeros_like(np.asarray(in0, np.float32))

    @dataclass(frozen=True)
    class _RawUopDveOp(DOPS.DveOp):
        def compile(self, ver):
            key = (self.name, ver)
            cached = DOPS._COMPILE_CACHE.get(key)
            if cached is not None:
                return cached
            if self.name == LDBH_OP:
                uops = _build_ldbh_uops()
            else:
                uops = _build_scan_uops(SCAN_OPS.index(self.name))
            spec = DveOpSpec(
                name=self.name,
                opcode=DOPS.get_dve_sub_opcode(self.name),
                uops=uops,
                rd1_en=(self.name != LDBH_OP),
            )
            spec.validate(ver)
            DOPS._COMPILE_CACHE[key] = spec
            return spec

    def _reg(name, ref, body):
        op = _RawUopDveOp(name=name, spec=Spec(body=body, reference=ref),
                          subdim=False, uops_sha={})
        DOPS.OPS.append(op)
        DOPS._SUB_OPCODE_FOR_NAME[name] = (
            DOPS._CUSTOM_DVE_ROW_BASE + len(DOPS.OPS) - 1)
        DOPS.CUSTOM_DVE_SPECS[name] = op.spec
        return op

    scans = [_reg(n, _scan_reference, Src0 + Src1 * C0) for n in SCAN_OPS]
    ld = _reg(LDBH_OP, _ld_reference, Src0 * C0)
    return scans, ld


def _build():
    import concourse.tile as tile
    from concourse import bacc, mybir

    f32 = mybir.dt.float32
    bf16 = mybir.dt.bfloat16
    fp8 = mybir.dt.float8e4
    AF = mybir.ActivationFunctionType
    PM = mybir.MatmulPerfMode.DoubleRow

    scan_ops, ld_op = _register_dve_ops()

    nc = bacc.Bacc("TRN2", target_bir_lowering=False, debug=False,
                   num_devices=N_CORES)

    xt_d = nc.dram_tensor("xt", [D, T], bf16, kind="ExternalInput").ap()
    xq_d = nc.dram_tensor("xq", [D, T], fp8, kind="ExternalInput").ap()
    wh_d = nc.dram_tensor("wh", [D, H], bf16, kind="ExternalInput").ap()
    wz_d = nc.dram_tensor("wz", [D, H], fp8, kind="ExternalInput").ap()
    # bias_d: [128, 3*NM] f32 = per m-tile columns [b_z | b_h | b_h+0.5]
    bias_d = nc.dram_tensor("bias", [128, 3 * NM], f32,
                            kind="ExternalInput").ap()
    ht_d = nc.dram_tensor("ht", [H, T], bf16, kind="ExternalOutput").ap()

    with tile.TileContext(nc) as tc:
        with (
            tc.tile_pool(name="const", bufs=1) as const,
            tc.tile_pool(name="zs", bufs=3) as zsp,
            tc.tile_pool(name="hp", bufs=6) as hp,
            tc.tile_pool(name="psT", bufs=2, space="PSUM") as psTp,
            tc.tile_pool(name="psK", bufs=2, space="PSUM") as psKp,
        ):
            # sync ring (HWDGE): wh, bias, then xt (bf16) in chunk order.
            # gpsimd ring (SWDGE): wz, xq (fp8) in chunk order, then the
            # f32->bf16 casting h stores.
            wh_s = const.tile([128, NK, H], bf16, tag="wh")
            nc.sync.dma_start(wh_s[:], wh_d.rearrange("(k p) h -> p k h", p=128))
            bias_s = const.tile([128, 3 * NM], f32, tag="bias")
            nc.sync.dma_start(bias_s[:], bias_d[:])
            wz_s = const.tile([128, NK, H], fp8, tag="wz")
            nc.gpsimd.dma_start(wz_s[:], wz_d.rearrange("(k p) h -> p k h", p=128))
            ldout_s = const.tile([128, NM], f32, tag="ldout")
            xt_s = const.tile([128, NK, T], bf16, tag="xt")
            xq_s = const.tile([128, NK, T], fp8, tag="xq")
            xt_r = xt_d.rearrange("(k p) t -> p k t", p=128)
            xq_r = xq_d.rearrange("(k p) t -> p k t", p=128)
            nc.sync.dma_start(xt_s[:, :, 0:MMN], xt_r[:, :, 0:MMN])
            nc.gpsimd.dma_start(xq_s[:, :, 0:MMN], xq_r[:, :, 0:MMN])
            nc.sync.dma_start(xt_s[:, :, MMN:TCH], xt_r[:, :, MMN:TCH])
            nc.gpsimd.dma_start(xq_s[:, :, MMN:TCH], xq_r[:, :, MMN:TCH])
            for tc_i in range(1, NT):
                tsl = slice(tc_i * TCH, (tc_i + 1) * TCH)
                nc.sync.dma_start(xt_s[:, :, tsl], xt_r[:, :, tsl])
                nc.gpsimd.dma_start(xq_s[:, :, tsl], xq_r[:, :, tsl])

            # load all four bh5 swap flops once (off the critical path)
            nc.vector._custom_dve(ld_op, out=ldout_s[:],
                                  in0=bias_s[:, 2 * NM:3 * NM])

            # PE warm-up during the wh/x DMA wait so the HAM clock gate is
            # ramping when real work arrives.
            warm = psKp.tile([128, TCH], f32, tag="psK")
            for r in range(8):
                nc.tensor.matmul(warm[:, 0:256], wh_s[:, 0, 0:128],
                                 wh_s[:, 0, 0:256], start=True, stop=True)

            h_prev = [None] * NM
            hp_last = [0] * NM

            def emit_chunk(m, t0, tlen, first):
                msl = slice(m * 128, (m + 1) * 128)
                psT = psTp.tile([128, TCH], f32, tag="psT")
                psK = psKp.tile([128, TCH], f32, tag="psK")
                for sub in range(tlen // MMN):
                    nsl = slice(t0 + sub * MMN, t0 + (sub + 1) * MMN)
                    osl = slice(sub * MMN, (sub + 1) * MMN)
                    for k in range(NK):
                        nc.tensor.matmul(psT[:, osl], wh_s[:, k, msl],
                                         xt_s[:, k, nsl],
                                         start=(k == 0), stop=(k == NK - 1))
                    for k in range(0, NK, 2):
                        nc.tensor.matmul(psK[:, osl], wz_s[:, k:k + 2, msl],
                                         xq_s[:, k:k + 2, nsl],
                                         start=(k == 0), stop=(k == NK - 2),
                                         perf_mode=PM)
                # zs[:,0,:] = z = sigmoid(psK/16 + b_z)
                # zs[:,1,:] = sg = sigmoid(psT + b_h)
                zs = zsp.tile([128, 2, TCH], bf16, tag="zs")
                nc.scalar.activation(zs[:, 1, 0:tlen], psT[:, 0:tlen],
                                     AF.Sigmoid,
                                     bias=bias_s[:, NM + m:NM + m + 1],
                                     scale=1.0)
                nc.scalar.activation(zs[:, 0, 0:tlen], psK[:, 0:tlen],
                                     AF.Sigmoid,
                                     bias=bias_s[:, m:m + 1], scale=1.0 / WSC)
                # fused scan: h[t] = (1-z)*h[t-1] + z*max(psT+bh5, sg)
                # (variant m reads bh5 from block-m's swap flop; f32 h so
                # the next chunk's s0 init AP is f32; the output DMA casts
                # to bf16 on the gpsimd ring)
                h = hp.tile([128, TCH], f32, tag="h")
                init = (ldout_s[:, m:m + 1] if first
                        else h_prev[m][:, hp_last[m]:hp_last[m] + 1])
                nc.vector._custom_dve(
                    scan_ops[m], out=h[:, 0:tlen],
                    in0=zs[:, :, 0:tlen].rearrange("p j t -> p t j"),
                    in1=psT[:, 0:tlen].unsqueeze(2).broadcast_to(
                        [128, tlen, 2]),
                    s0=init)
                nc.gpsimd.dma_start(ht_d[msl, t0:t0 + tlen], h[:, 0:tlen])
                h_prev[m] = h
                hp_last[m] = tlen - 1

            # tc group 0 runs as 512-wide half chunks: the first scans
            # start ~half a chunk of (cold-clock) PE work earlier.
            for half in range(2):
                for m in range(NM):
                    emit_chunk(m, half * MMN, MMN, first=(half == 0))
            for tc_i in range(1, NT):
                for m in range(NM):
                    emit_chunk(m, tc_i * TCH, TCH, first=False)

    nc.compile()
    return nc


def kernel(x, W_z, b_z, W_h, b_h):
    import ml_dtypes
    from concourse.bass_utils import run_bass_kernel_spmd

    if "nc" not in _cache:
        _cache["nc"] = _build()
    nc = _cache["nc"]

    bf = ml_dtypes.bfloat16
    f8 = ml_dtypes.float8_e4m3
    x = np.asarray(x, dtype=np.float32)
    W_z = np.asarray(W_z, dtype=np.float32)
    W_h = np.asarray(W_h, dtype=np.float32)
    b_z = np.asarray(b_z, dtype=np.float32)
    b_h = np.asarray(b_h, dtype=np.float32)

    wh = np.ascontiguousarray(W_h.astype(bf))
    wz = np.ascontiguousarray((W_z * WSC).astype(f8))
    bias = np.ascontiguousarray(np.concatenate(
        [b_z.reshape(NM, 128).T, b_h.reshape(NM, 128).T,
         (b_h + 0.5).reshape(NM, 128).T], axis=1).astype(np.float32))

    in_maps = []
    for b in range(B):
        xt = x[b].T
        in_maps.append({
            "xt": np.ascontiguousarray(xt.astype(bf)),
            "xq": np.ascontiguousarray(xt.astype(f8)),
            "wh": wh,
            "wz": wz,
            "bias": bias,
        })

    import os
    kwargs = {}
    if os.environ.get("KERNEL_TRACE"):
        kwargs = dict(trace=True, tmpdir=os.environ.get("KERNEL_TMPDIR"))
    try:
        res = run_bass_kernel_spmd(nc, in_maps, core_ids=list(range(N_CORES)),
                                   **kwargs)
    except Exception:
        # transient accelerator errors recover on retry
        res = run_bass_kernel_spmd(nc, in_maps, core_ids=list(range(N_CORES)),
                                   **kwargs)
    _cache["last_results"] = res

    out = np.empty((B, T, H), dtype=np.float32)
    for b in range(B):
        out[b] = np.asarray(res.results[b]["ht"]).astype(np.float32).T
    return out
